# revision 18
# baseline (speedup 1.0000x reference)
"""Masked dot-product attention on 8 Trainium2 NeuronCores.

Problem: q,k,v [64, 1024, 64] f32, valid_lens [64] int32.
  scores = q @ k^T / 8, mask keys >= valid_len to -1e6, softmax, @ v.

Strategy (per core: 8 batches, pure data parallelism, no collectives):
  - Host prep: pre-transpose q,k to [D, S] (contraction dim on partitions),
    pre-zero v rows past valid_len and append the 0/1 mask as a 65th column
    (vm).  The masked softmax denominator then falls out of the same matmul
    that computes attn @ v.  valid_len==0 batches reproduce the reference's
    uniform-softmax by zeroing q (scores==0) and unmasking all keys.
  - Matmul dtypes, chosen off measured TRN2 PE rates: scores run f32r x f32r
    (1.5 cycles/row, near-fp32 accuracy); the attn@v stream runs fp16
    (2 cycles/row; bf16 would be 1 cycle but its 8-bit mantissa costs ~2e-3
    output error, fp16 ~3e-4).
  - Device, per key-tile j: scoresT[j,q] = kT_tile.T.T @ qT (keys on
    partitions, fp32 PSUM), exp on ScalarE (scale=1/8, bias=-3 bounds the
    fp16 range; numerator and denominator share it so it cancels), then
    po[65, Sq] += vm_tile.T.T @ expT accumulated over j in fp32 PSUM.
    No max-subtraction: scores are O(1) (q,k ~ N(0,1), d=64), and masked
    columns are excluded by the mask column/zeroed v rows, not by -1e6.
  - Transpose-free epilogue: reciprocal of the denominator row (PSUM ->
    SBUF), bounce it through a DRAM scratch tile to broadcast across 64
    partitions (DMA with 0-stride source), one tensor_tensor multiply
    normalizes the [64, Sq] block, DMA out in [d, q] layout; the host
    transposes each batch during the gather.
  - DMA dispatch is the hidden serial resource (~0.6us per dma_start on a
    sequencer): inputs ride the Sync queue, epilogue DMAs the GpSimd queue.
  - Per-batch key tiles are truncated to ceil(valid/128): masked tail tiles
    contribute exactly zero, so they are skipped.  Batches are rank-sorted by
    valid_len and dealt one per core per slot so every core runs the same
    baked schedule with minimal padding.
"""

import numpy as np

import concourse.bass as bass
import concourse.bacc as bacc
import concourse.tile as tile
from concourse import mybir
from concourse import bass_utils

B, S, D = 64, 1024, 64
NCORES = 8
NB = B // NCORES  # batch slots per core
P = 128
NJT = S // P  # max key tiles per batch
W = D + 1  # v columns + mask column
F32 = mybir.dt.float32
F32R = mybir.dt.float32r
F16 = mybir.dt.float16

TRACE = False  # set by test harness to capture an NTFF profile
LAST_RESULTS = None  # BassKernelResults stash for the harness

_program_cache = {}


def _build_program(jt_counts):
    nc = bacc.Bacc("TRN2", target_bir_lowering=False, debug=False,
                   num_devices=NCORES)
    qT = nc.dram_tensor("qT", [NB, D, S], F32R, kind="ExternalInput").ap()
    kT = nc.dram_tensor("kT", [NB, D, S], F32R, kind="ExternalInput").ap()
    vm = nc.dram_tensor("vm", [NB, S, W], F16, kind="ExternalInput").ap()
    out = nc.dram_tensor("outT", [NB, D, S], F32, kind="ExternalOutput").ap()

    with tile.TileContext(nc) as tc:
        with (
            tc.tile_pool(name="singles", bufs=1) as singles,
            tc.tile_pool(name="qk", bufs=2) as qk_pool,
            tc.tile_pool(name="vmp", bufs=2) as vm_pool,
            tc.tile_pool(name="ex", bufs=4) as ex_pool,
            tc.tile_pool(name="rcp", bufs=2) as rcp_pool,
            tc.tile_pool(name="rbc", bufs=2) as rbc_pool,
            tc.tile_pool(name="osb", bufs=2) as osb_pool,
            tc.tile_pool(name="scr", bufs=2, space="DRAM") as scr_pool,
            tc.tile_pool(name="ps_s", bufs=2, space="PSUM") as ps_pool,
            tc.tile_pool(name="ps_o", bufs=2, space="PSUM") as po_pool,
        ):
            # exp(s/8 - 3): the -3 bounds the fp16 exp range; it cancels
            # between numerator and denominator.
            bias_t = singles.tile([P, 1], F32)
            nc.vector.memset(bias_t, -3.0)

            for s in range(NB):
                jt = jt_counts[s]
                qT_t = qk_pool.tile([D, S], F32R, tag="qT")
                kT_t = qk_pool.tile([D, S], F32R, tag="kT")
                nc.sync.dma_start(out=qT_t, in_=qT[s])
                nc.sync.dma_start(out=kT_t[:, 0:jt * P], in_=kT[s, :, 0:jt * P])
                # All key tiles of vm in one DMA: [128, jt*65], tile j at
                # columns [j*65, (j+1)*65).
                vm_t = vm_pool.tile([P, NJT * W], F16, tag="vm", name="vm_t")
                nc.sync.dma_start(
                    out=vm_t.rearrange("p (j w) -> p j w", w=W)[:, 0:jt, :],
                    in_=vm[s, 0:jt * P, :].rearrange("(j p) w -> p j w", p=P),
                )
                # [v|mask]^T-weighted sums: rows 0..63 unnormalized outT,
                # row 64 the softmax denominator.  One accumulation group
                # per PSUM bank (cols 0:512 and 512:1024), spanning all j.
                po = po_pool.tile([W, S], F32, name="po")

                def emit_av(ex_j, j, jt=jt, po=po, vm_t=vm_t):
                    for half in range(2):
                        nc.tensor.matmul(
                            po[:, half * 512:(half + 1) * 512],
                            lhsT=vm_t[:, j * W:(j + 1) * W],
                            rhs=ex_j[:, half * 512:(half + 1) * 512],
                            start=(j == 0), stop=(j == jt - 1),
                        )

                # Scores/exp run one j ahead of the attn@v accumulation so
                # the PE never sits behind ScalarE in its own queue.
                prev = None
                for j in range(jt):
                    ps = ps_pool.tile([P, S], F32, tag="ps")
                    for half in range(2):
                        nc.tensor.matmul(
                            ps[:, half * 512:(half + 1) * 512],
                            lhsT=kT_t[:, j * P:(j + 1) * P],
                            rhs=qT_t[:, half * 512:(half + 1) * 512],
                            start=True, stop=True,
                        )
                    ex = ex_pool.tile([P, S], F16, tag="ex", name="ex")
                    nc.scalar.activation(out=ex, in_=ps,
                                         func=mybir.ActivationFunctionType.Exp,
                                         scale=0.125, bias=bias_t)
                    if prev is not None:
                        emit_av(*prev)
                    prev = (ex, j)
                emit_av(*prev)

                # 1/denominator, then broadcast across the 64 partitions by
                # bouncing through DRAM (0-stride partition reads are only
                # legal on DRAM sources).
                rcp = rcp_pool.tile([1, S], F32, tag="rcp", name="rcp")
                nc.vector.reciprocal(out=rcp, in_=po[D:W, :])
                scr = scr_pool.tile([1, S], F32, tag="scr", name="scr")
                nc.gpsimd.dma_start(out=scr, in_=rcp)
                rbc = rbc_pool.tile([D, S], F32, tag="rbc", name="rbc")
                bcast_src = bass.AP(tensor=scr.tensor, offset=scr.offset,
                                    ap=[[0, D], scr.ap[-1]])
                nc.gpsimd.dma_start(out=rbc, in_=bcast_src)
                osb = osb_pool.tile([D, S], F32, tag="osb", name="osb")
                nc.vector.tensor_mul(osb, po[0:D, :], rbc)
                nc.gpsimd.dma_start(out=out[s], in_=osb)
    nc.compile()
    return nc


def kernel(q, k, v, valid_lens):
    global LAST_RESULTS
    q = np.array(q, dtype=np.float32, copy=True)
    k = np.asarray(k, dtype=np.float32)
    v = np.asarray(v, dtype=np.float32)
    vl = np.asarray(valid_lens).astype(np.int64)

    # valid_len == 0: reference's softmax over an all-masked row is uniform.
    # Zeroed q gives scores == 0 -> exp == 1 over all (unmasked) keys: same.
    valid_eff = np.where(vl <= 0, S, np.minimum(vl, S))
    q[vl <= 0] = 0.0

    mask = (np.arange(S)[None, :] < valid_eff[:, None]).astype(np.float32)
    qT = np.ascontiguousarray(q.transpose(0, 2, 1))
    kT = np.ascontiguousarray(k.transpose(0, 2, 1))
    vm = np.concatenate([v * mask[:, :, None], mask[:, :, None]], axis=2)
    vm = np.ascontiguousarray(vm).astype(np.float16)

    # Rank-sort batches by effective length; slot s takes ranks [8s, 8s+8),
    # one per core, so the baked per-slot tile count wastes little work.
    order = np.argsort(-valid_eff, kind="stable")
    assign = order.reshape(NB, NCORES)  # [slot, core] -> batch index
    jt_counts = tuple(
        int(np.ceil(valid_eff[assign[s]].max() / P)) for s in range(NB)
    )

    nc = _program_cache.get(jt_counts)
    if nc is None:
        nc = _build_program(jt_counts)
        _program_cache[jt_counts] = nc

    in_maps = []
    for c in range(NCORES):
        bs = assign[:, c]
        in_maps.append({
            "qT": np.ascontiguousarray(qT[bs]),
            "kT": np.ascontiguousarray(kT[bs]),
            "vm": np.ascontiguousarray(vm[bs]),
        })
    res = bass_utils.run_bass_kernel_spmd(
        nc, in_maps, core_ids=list(range(NCORES)), trace=TRACE,
    )
    LAST_RESULTS = res

    out = np.empty((B, S, D), dtype=np.float32)
    for c in range(NCORES):
        o = res.results[c]["outT"]  # [NB, D, S]
        for s in range(NB):
            out[assign[s, c]] = o[s].T
    return out
